# revision 22
# baseline (speedup 1.0000x reference)
"""DMPNet Trainium2 kernel.

Strategy
--------
* Pure batch data parallelism: 16384 rows -> 8 cores x 2048.
* The MLP (128 -> 2048 -> 2048 -> 54, tanh) runs on the tensor engine
  entirely in bf16 operands with fp32 PSUM accumulation (~4.8e-3 rel
  err, gate is 2e-2).  bf16 halves SBUF + weight DMA and gets the
  fast-weight-load path on every matmul; all matmuls stream 1 col/cycle.
* The 101-step DMP Euler integration is a linear time-invariant recurrence
  in (y, z); it collapses exactly into
      out[r, j] = da_j*y0 + db_j*dy0 + dg_j*goal + (goal - y0) * (w @ dQ_j)
  with coefficients precomputed on the host in float64.  The (w @ dQ) part
  is folded into the final-layer weights (W_eff), so the device only runs
  3 matmul layers + 2 tiny broadcast matmuls + 2 elementwise ops.
* All activations live feature-major ([feature, batch]); the input is
  transposed host-side and staged ONCE into SBUF (x_all) - the steady-state
  pass has zero input DMAs.  Layer-0 chunks for the next batch tile are
  spread one-per-j through the layer-1 j-loop so the scalar engine's tanh
  throughput never gates the PE.  h0 lives in two persistent 16-tile
  generations that ping-pong across batch tiles (and across For_i
  iterations for the timing loop).
* The PE executes its queue strictly in order, so any matmul that consumes
  an ACT/DVE product is issued with SLACK: the weff matmul for group (t,j)
  is delayed into the middle of group j+1 (WEFF_AT), the m2 bias-add and
  the tail matmuls one group further (BIAS_AT/TAIL_AT).  The pipeline
  state carries across passes in the straight-line repeat path so a pass's
  trailing weff/bias/tail overlap the next pass's first groups.
* Measured floor: ~195 ns per N=512 bf16 matmul in tiny (<=1 IRAM block)
  For_i bodies; long instruction streams (loop or straight-line) pay a
  ~55-65 ns/MM instruction-streaming tax that is structural here (every
  weight block needs its own static LDWEIGHTS+MATMUL; stationary APs
  cannot be register-indexed), putting this kernel's ~1160 MM/pass at a
  ~300 us practical floor, which it achieves.
"""

import os

import ml_dtypes
import numpy as np

import concourse.bass as bass
import concourse.mybir as mybir
from concourse import bacc
from concourse.tile import TileContext
from concourse.bass_utils import run_bass_kernel_spmd

F32 = mybir.dt.float32
F32R = mybir.dt.float32r
BF16 = mybir.dt.bfloat16

N_CORES = 8
B_TOTAL = 16384
B_SH = B_TOTAL // N_CORES          # 2048 rows per core
D_IN = 128
H = 2048
HC = H // 128                      # 16 chunks of 128
DIM = 9
N_BASIS = 5
NOUT = 10                          # output time steps
M_S = DIM * NOUT                   # 90 "S" rows
M_ALL = M_S + DIM                  # 99 rows of the effective final layer

TW = int(os.environ.get("DMP_TW", "512"))            # batch tile width
REPEAT = int(os.environ.get("DMP_KERNEL_REPEAT", "1"))
FORI_REPS = int(os.environ.get("DMP_FORI_REPS", "1"))  # hardware-loop reps (timing)
STAGGER = int(os.environ.get("DMP_STAGGER", "1"))     # staggered-reset For_i
NT = B_SH // TW
# software-pipeline insert positions (i-slot within the next j-group where a
# delayed dependent op is issued; gives the ACT/DVE producer time to drain
# so the in-order PE queue never stalls on it)
WEFF_AT = int(os.environ.get("DMP_WEFF_AT", "4"))
BIAS_AT = int(os.environ.get("DMP_BIAS_AT", "6"))
TAIL_AT = int(os.environ.get("DMP_TAIL_AT", "8"))
L0_AT = int(os.environ.get("DMP_L0_AT", "12"))
L1_ONLY = int(os.environ.get("DMP_L1_ONLY", "0"))  # debug: time bare L1 loop

_TANH = mybir.ActivationFunctionType.Tanh


def _round_fp32r(x: np.ndarray) -> np.ndarray:
    """Round fp32 -> fp32r (11 explicit mantissa bits), nearest-even."""
    b = np.ascontiguousarray(x, dtype=np.float32).view(np.uint32)
    lsb = (b >> np.uint32(12)) & np.uint32(1)
    r = b + (np.uint32(0x7FF) + lsb)
    r &= np.uint32(0xFFFFF000)
    return r.view(np.float32)


def _bf16(x: np.ndarray) -> np.ndarray:
    return np.ascontiguousarray(np.asarray(x, np.float32)).astype(ml_dtypes.bfloat16)


def _dmp_coefficients():
    """Closed-form coefficients of the sampled-position differences.

    Returns (d_alpha, d_beta, d_gamma, dQ) with dQ shaped (NOUT, N_BASIS):
      out[r, j] = d_alpha[j]*y0 + d_beta[j]*dy0 + d_gamma[j]*goal
                  + (goal - y0) * sum_n w[r, n] * dQ[j, n]
    """
    A_X, A_Z, TAU, DT = 1.0, 25.0, 1.0, 0.01
    B_Z = A_Z / 4.0
    NSTEP, L_SUB = 100, 10

    c = np.exp(-A_X * np.linspace(0.0, 1.0, N_BASIS))
    h = N_BASIS ** 1.5 / c / A_X
    xs = (1.0 - A_X * DT / TAU) ** np.arange(1, NSTEP + 1)
    psi = np.exp(-h[None, :] * (xs[:, None] - c[None, :]) ** 2)
    p = psi * xs[:, None] / psi.sum(axis=1, keepdims=True)      # (100, 5)

    nb = 3 + NSTEP
    cy = np.zeros(nb)
    cz = np.zeros(nb)
    cy[0] = 1.0
    cz[1] = TAU
    ys = [cy.copy()]
    for k in range(NSTEP):
        dz = np.zeros(nb)
        dz[2] = A_Z * B_Z
        dz -= A_Z * B_Z * cy
        dz -= A_Z * cz
        dz[3 + k] += 1.0
        dz /= TAU
        dy = cz / TAU
        cy = cy + dy * DT
        cz = cz + dz * DT
        ys.append(cy.copy())
    ys = np.array(ys)                         # (101, 103)
    samp = ys[::L_SUB]                        # (11, 103)
    dcoef = samp[1:] - samp[:-1]              # (10, 103)
    dQ = dcoef[:, 3:] @ p                     # (10, 5)
    return dcoef[:, 0], dcoef[:, 1], dcoef[:, 2], dQ


_NC_CACHE = {}


def _build_program(tw: int, repeat: int, fori_reps: int = 1, stagger: int = STAGGER):
    nt = B_SH // tw
    # one h0 generation per batch tile, written two tiles ahead; the
    # staggered-reset stage-adjacency invariant (stage I waits on I-2)
    # makes the cross-iteration handoff race-free only at distance 2
    assert nt == 4, f"nt={nt} must be 4"
    nc = bacc.Bacc()

    xb = nc.dram_tensor("xb", [D_IN, B_SH], BF16, kind="ExternalInput")
    ydy = nc.dram_tensor("ydy", [18, B_SH], BF16, kind="ExternalInput")
    w0t = nc.dram_tensor("w0t", [D_IN, H], BF16, kind="ExternalInput")
    b0d = nc.dram_tensor("b0d", [128, HC], F32, kind="ExternalInput")
    w1t = nc.dram_tensor("w1t", [H, H], BF16, kind="ExternalInput")
    b1d = nc.dram_tensor("b1d", [128, HC], F32, kind="ExternalInput")
    weff = nc.dram_tensor("weff", [H, M_ALL], BF16, kind="ExternalInput")
    beff = nc.dram_tensor("beff", [M_ALL, 1], F32, kind="ExternalInput")
    linc = nc.dram_tensor("linc", [117, M_S], BF16, kind="ExternalInput")
    diffc = nc.dram_tensor("diffc", [117, M_S], BF16, kind="ExternalInput")
    outT = nc.dram_tensor("outT", [M_S, B_SH], F32, kind="ExternalOutput")

    with TileContext(nc) as tc:
        with (
            tc.tile_pool(name="wres", bufs=1) as wres,
            tc.tile_pool(name="h1p", bufs=3) as h1p,
            tc.tile_pool(name="outp", bufs=4) as outp,
            tc.tile_pool(name="ps_l0", bufs=2, space="PSUM") as ps_l0,
            tc.tile_pool(name="ps_h1", bufs=4, space="PSUM") as ps_h1,
            tc.tile_pool(name="ps_m", bufs=2, space="PSUM") as ps_m,
        ):
            # ---- resident weights / constants / input ----
            w0_sb = wres.tile([128, H], BF16, tag="w0")
            nc.sync.dma_start(out=w0_sb, in_=w0t[:, :])
            b0_sb = wres.tile([128, HC], F32, tag="b0")
            nc.sync.dma_start(out=b0_sb, in_=b0d[:, :])
            x_all = wres.tile([128, B_SH], BF16, tag="xall")
            nc.sync.dma_start(out=x_all, in_=xb[:, :])
            b1_sb = wres.tile([128, HC], F32, tag="b1")
            nc.sync.dma_start(out=b1_sb, in_=b1d[:, :])
            # j-major 128x128 blocks: the layer-1 j-loop consumes w1[:, :, j*128]
            # column blocks in order, so tile 0's j-loop can start as soon as
            # the first blocks land instead of waiting for the whole 8 MB
            w1_sb = wres.tile([128, HC, H], BF16, tag="w1")
            for j in range(HC):
                for i in range(HC):
                    nc.sync.dma_start(
                        out=w1_sb[:, i, j * 128:(j + 1) * 128],
                        in_=w1t[i * 128:(i + 1) * 128, j * 128:(j + 1) * 128],
                    )
            weff_sb = wres.tile([128, HC, M_ALL], BF16, tag="weff")
            for i in range(HC):
                nc.sync.dma_start(out=weff_sb[:, i, :], in_=weff[i * 128:(i + 1) * 128, :])
            beff_sb = wres.tile([M_ALL, 1], F32, tag="beff")
            nc.sync.dma_start(out=beff_sb, in_=beff[:, :])
            linc_sb = wres.tile([117, M_S], BF16, tag="linc")
            nc.sync.dma_start(out=linc_sb, in_=linc[:, :])
            diffc_sb = wres.tile([117, M_S], BF16, tag="diffc")
            nc.sync.dma_start(out=diffc_sb, in_=diffc[:, :])

            # persistent h0 double generation + per-tile m2 combine tiles
            h0g = [
                [
                    wres.tile(
                        [128, tw], BF16, tag=f"h0_{g}_{c}", name=f"h0_{g}_{c}"
                    )
                    for c in range(HC)
                ]
                for g in range(nt)
            ]
            m2t = [
                wres.tile([117, tw], BF16, tag=f"m2_{t}", name=f"m2_{t}")
                for t in range(nt)
            ]
            for t in range(nt):
                win = slice(t * tw, (t + 1) * tw)
                nc.sync.dma_start(out=m2t[t][99:117, :], in_=ydy[:, win])

            def l0_mm(t_target, c):
                """Layer-0 matmul for chunk c of batch tile t_target."""
                win = slice(t_target * tw, (t_target + 1) * tw)
                ps = ps_l0.tile([128, tw], F32, tag="l0")
                nc.tensor.matmul(
                    ps, w0_sb[:, c * 128:(c + 1) * 128], x_all[:, win],
                    start=True, stop=True,
                )
                return ps

            def l0_act(ps, dst, c):
                nc.scalar.activation(
                    out=dst, in_=ps, func=_TANH, bias=b0_sb[:, c:c + 1],
                )

            def l0_chunk(dst, t_target, c):
                """h0 chunk c for batch tile t_target -> persistent tile dst."""
                l0_act(l0_mm(t_target, c), dst, c)

            # preamble: generations 0/1 = tiles 0/1's h0
            for c in range(HC):
                l0_chunk(h0g[0][c], 0, c)
            for c in range(HC):
                l0_chunk(h0g[1][c], 1, c)
            if L1_ONLY:
                for g in (2, 3):
                    for c in range(HC):
                        l0_chunk(h0g[g][c], g, c)

            def tail(t):
                win = slice(t * tw, (t + 1) * tw)
                m2 = m2t[t]
                lin_ps = ps_l0.tile([M_S, tw], F32, tag="l0")
                nc.tensor.matmul(lin_ps, linc_sb, m2[0:117, :], start=True, stop=True)
                diff_ps = ps_l0.tile([M_S, tw], F32, tag="l0")
                nc.tensor.matmul(diff_ps, diffc_sb, m2[0:117, :], start=True, stop=True)
                prod = outp.tile([M_S, tw], F32, tag="prod")
                nc.vector.tensor_mul(prod, diff_ps, m2[0:M_S, :])
                res = outp.tile([M_S, tw], F32, tag="res")
                nc.vector.tensor_add(res, prod, lin_ps)
                nc.sync.dma_start(out=outT[:, win], in_=res)

            # software-pipeline state; carried across passes in the
            # straight-line repeat path so a pass's trailing weff/bias/tail
            # overlap the next pass's first groups (flushed per-iteration in
            # the For_i path, which must be self-contained)
            psm_tiles = {}
            pend_weff = []   # (t, j, h1c) awaiting weff issue
            pend_bias = []   # tiles whose psm group closed, bias not issued
            pend_tail = []   # tiles whose bias issued, tail not issued

            def _one_pass(staged=False, flush=True, boundaries=None, g0=0):
                # boundaries: set of global tile indices that begin a new
                # stagger stage (the reset machinery needs exactly 4 stages
                # per For_i body); g0 = this pass's first global tile index
                # software pipeline: the weff matmul for group (t, j) is
                # issued WEFF_AT matmuls into group j+1 so the in-order PE
                # queue never waits on the ACT tanh that produces h1c.
                # Likewise the m2 bias-add (DVE) and the tail matmuls.

                def issue_weff():
                    t_, j_, h1c_ = pend_weff.pop(0)
                    if j_ == 0:
                        psm_tiles[t_] = ps_m.tile(
                            [M_ALL, tw], F32, tag="m", name=f"psm_{t_}"
                        )
                    nc.tensor.matmul(
                        psm_tiles[t_], weff_sb[:, j_, :], h1c_,
                        start=(j_ == 0), stop=(j_ == HC - 1),
                        skip_group_check=True,
                    )
                    if j_ == HC - 1:
                        pend_bias.append(t_)

                def issue_bias():
                    t_ = pend_bias.pop(0)
                    nc.vector.tensor_scalar_add(
                        out=m2t[t_][0:M_ALL, :],
                        in0=psm_tiles.pop(t_),
                        scalar1=beff_sb[:, 0:1],
                    )
                    pend_tail.append(t_)

                for t in range(nt):
                    if staged and (boundaries is None or (g0 + t) in boundaries):
                        if boundaries is not None or t > 0:
                            tc.stage_boundary()
                    gen = h0g[t]
                    ngen = h0g[(t + 2) % nt]
                    nxt = (t + 2) % nt
                    for j in range(HC):
                        ps1 = ps_h1.tile([128, tw], F32, tag="h1")
                        for i in range(HC):
                            nc.tensor.matmul(
                                ps1, w1_sb[:, i, j * 128:(j + 1) * 128], gen[i],
                                start=(i == 0), stop=(i == HC - 1),
                            )
                            if L1_ONLY:
                                continue
                            if i == WEFF_AT and pend_weff:
                                issue_weff()
                            if i == BIAS_AT and pend_bias:
                                issue_bias()
                            if i == TAIL_AT and pend_tail and not (t == 0 and j == 0):
                                tail(pend_tail.pop(0))
                            if i == L0_AT and not L1_ONLY:
                                # layer-0 matmul for tile t+2 (wraps into the
                                # next pass for t >= 2; x is identical every
                                # pass); its tanh is issued AFTER h1c's so
                                # the strict-FIFO ACT queue runs h1c(j)
                                # first — the weff matmul needs h1c ~4
                                # matmuls into group j+1, while the l0 tanh
                                # output isn't read until two tiles later
                                l0_ps = l0_mm(nxt, j)
                        h1c = h1p.tile([128, tw], BF16, tag="h1c")
                        nc.scalar.activation(
                            out=h1c, in_=ps1, func=_TANH, bias=b1_sb[:, j:j + 1],
                        )
                        if L1_ONLY:
                            continue
                        pend_weff.append((t, j, h1c))
                        l0_act(l0_ps, ngen[j], j)
                # end-of-pass flush (last tile's final weff, bias, tail);
                # skipped mid-stream in the repeat path so the next pass's
                # first groups absorb the latency
                if flush:
                    while pend_weff:
                        issue_weff()
                    while pend_bias:
                        issue_bias()
                    while pend_tail:
                        tail(pend_tail.pop(0))

            if fori_reps > 1:
                ntiles = repeat * nt
                assert ntiles % 4 == 0
                q = ntiles // 4
                bset = {q, 2 * q, 3 * q}
                with tc.For_i(
                    0, fori_reps, 1,
                    hint_engines=(mybir.EngineType.PE,),
                    staggered_reset=bool(stagger),
                ):
                    for _rep in range(repeat):
                        _one_pass(
                            staged=bool(stagger),
                            flush=(_rep == repeat - 1),
                            boundaries=bset,
                            g0=_rep * nt,
                        )
            else:
                for _rep in range(repeat):
                    _one_pass(flush=(_rep == repeat - 1))

    nc.compile()
    return nc


def _get_program(tw: int = TW, repeat: int = REPEAT, fori_reps: int = FORI_REPS):
    key = (tw, repeat, fori_reps, STAGGER)
    if key not in _NC_CACHE:
        _NC_CACHE[key] = _build_program(tw, repeat, fori_reps, STAGGER)
    return _NC_CACHE[key]


def _prepare_host_inputs(input, W0, b0, W1, b1, Wl, bl):
    """Build the per-core input maps (host-side prep, float64 coefficients)."""
    input, W0, b0, W1, b1, Wl, bl = (
        np.asarray(a) for a in (input, W0, b0, W1, b1, Wl, bl)
    )
    d_alpha, d_beta, d_gamma, dQ = _dmp_coefficients()

    Wl100 = Wl.astype(np.float64) * 100.0          # (54, H)
    bl100 = bl.astype(np.float64) * 100.0          # (54,)

    # effective final layer: rows 0..89 = S rows (d*10+j), 90..98 = goal rows
    weff = np.zeros((H, M_ALL), dtype=np.float64)
    beff = np.zeros((M_ALL,), dtype=np.float64)
    for d in range(DIM):
        for j in range(NOUT):
            m = d * NOUT + j
            wrow = np.zeros(H, dtype=np.float64)
            brow = 0.0
            for n in range(N_BASIS):
                wrow += dQ[j, n] * Wl100[DIM + N_BASIS * d + n]
                brow += dQ[j, n] * bl100[DIM + N_BASIS * d + n]
            weff[:, m] = wrow
            beff[m] = brow
        weff[:, M_S + d] = Wl100[d]
        beff[M_S + d] = bl100[d]

    # broadcast matmul constants [117, 90]: rhs is the combined mlp2 tile
    # (rows 0..89 = S [zero coeff], 90..98 = goal, 99..107 = y0, 108..116 = dy0)
    linc = np.zeros((117, M_S), dtype=np.float64)
    diffc = np.zeros((117, M_S), dtype=np.float64)
    for d in range(DIM):
        for j in range(NOUT):
            m = d * NOUT + j
            linc[90 + d, m] = d_gamma[j]
            linc[99 + d, m] = d_alpha[j]
            linc[108 + d, m] = d_beta[j]
            diffc[90 + d, m] = 1.0
            diffc[99 + d, m] = -1.0

    shared = {
        "w0t": _bf16(W0.T),
        "b0d": np.ascontiguousarray(np.asarray(b0, np.float32).reshape(HC, 128).T),
        "w1t": _bf16(W1.T),
        "b1d": np.ascontiguousarray(np.asarray(b1, np.float32).reshape(HC, 128).T),
        "weff": _bf16(weff),
        "beff": np.ascontiguousarray(beff.astype(np.float32).reshape(M_ALL, 1)),
        "linc": _bf16(linc),
        "diffc": _bf16(diffc),
    }

    x32 = np.asarray(input, np.float32)
    in_maps = []
    for c in range(N_CORES):
        m = dict(shared)
        xc = x32[c * B_SH:(c + 1) * B_SH, :]
        m["xb"] = _bf16(xc.T)
        m["ydy"] = _bf16(np.concatenate([xc[:, 7:16], xc[:, 22:31]], axis=1).T)
        in_maps.append(m)
    return in_maps


def kernel(input, W0, b0, W1, b1, Wl, bl):
    nc = _get_program()
    in_maps = _prepare_host_inputs(input, W0, b0, W1, b1, Wl, bl)
    results = run_bass_kernel_spmd(nc, in_maps, core_ids=list(range(N_CORES)))
    outs = []
    for c in range(N_CORES):
        o = results.results[c]["outT"]                     # (90, 2048)
        outs.append(o.reshape(DIM, NOUT, B_SH).transpose(2, 0, 1))
    return np.ascontiguousarray(np.concatenate(outs, axis=0), dtype=np.float32)



# revision 23
# speedup vs baseline: 1.0303x; 1.0303x over previous
"""DMPNet Trainium2 kernel.

Strategy
--------
* Pure batch data parallelism: 16384 rows -> 8 cores x 2048.
* The MLP (128 -> 2048 -> 2048 -> 54, tanh) runs on the tensor engine
  entirely in bf16 operands with fp32 PSUM accumulation (~4.8e-3 rel
  err, gate is 2e-2).  bf16 halves SBUF + weight DMA and gets the
  fast-weight-load path on every matmul; all matmuls stream 1 col/cycle.
* The 101-step DMP Euler integration is a linear time-invariant recurrence
  in (y, z); it collapses exactly into
      out[r, j] = da_j*y0 + db_j*dy0 + dg_j*goal + (goal - y0) * (w @ dQ_j)
  with coefficients precomputed on the host in float64.  The (w @ dQ) part
  is folded into the final-layer weights (W_eff), so the device only runs
  3 matmul layers + 2 tiny broadcast matmuls + 2 elementwise ops.
* All activations live feature-major ([feature, batch]); the input is
  transposed host-side and staged ONCE into SBUF (x_all) - the steady-state
  pass has zero input DMAs.  Layer-0 chunks for the next batch tile are
  spread one-per-j through the layer-1 j-loop so the scalar engine's tanh
  throughput never gates the PE.  h0 lives in two persistent 16-tile
  generations that ping-pong across batch tiles (and across For_i
  iterations for the timing loop).
* The PE executes its queue strictly in order, so any matmul that consumes
  an ACT/DVE product is issued with SLACK: the weff matmul for group (t,j)
  is delayed into the middle of group j+1 (WEFF_AT), the m2 bias-add and
  the tail matmuls one group further (BIAS_AT/TAIL_AT).  The pipeline
  state carries across passes in the straight-line repeat path so a pass's
  trailing weff/bias/tail overlap the next pass's first groups.
* Measured floor: ~195 ns per N=512 bf16 matmul in tiny (<=1 IRAM block)
  For_i bodies; long instruction streams (loop or straight-line) pay a
  ~55-65 ns/MM instruction-streaming tax that is structural here (every
  weight block needs its own static LDWEIGHTS+MATMUL; stationary APs
  cannot be register-indexed), putting this kernel's ~1160 MM/pass at a
  ~300 us practical floor, which it achieves.
"""

import os

import ml_dtypes
import numpy as np

import concourse.bass as bass
import concourse.mybir as mybir
from concourse import bacc
from concourse.tile import TileContext
from concourse.bass_utils import run_bass_kernel_spmd

F32 = mybir.dt.float32
F32R = mybir.dt.float32r
BF16 = mybir.dt.bfloat16

N_CORES = 8
B_TOTAL = 16384
B_SH = B_TOTAL // N_CORES          # 2048 rows per core
D_IN = 128
H = 2048
HC = H // 128                      # 16 chunks of 128
DIM = 9
N_BASIS = 5
NOUT = 10                          # output time steps
M_S = DIM * NOUT                   # 90 "S" rows
M_ALL = M_S + DIM                  # 99 rows of the effective final layer

TW = int(os.environ.get("DMP_TW", "512"))            # batch tile width
REPEAT = int(os.environ.get("DMP_KERNEL_REPEAT", "1"))
FORI_REPS = int(os.environ.get("DMP_FORI_REPS", "1"))  # hardware-loop reps (timing)
STAGGER = int(os.environ.get("DMP_STAGGER", "1"))     # staggered-reset For_i
NT = B_SH // TW
# software-pipeline insert positions (i-slot within the next j-group where a
# delayed dependent op is issued; gives the ACT/DVE producer time to drain
# so the in-order PE queue never stalls on it)
WEFF_AT = int(os.environ.get("DMP_WEFF_AT", "4"))
BIAS_AT = int(os.environ.get("DMP_BIAS_AT", "6"))
TAIL_AT = int(os.environ.get("DMP_TAIL_AT", "8"))
L0_AT = int(os.environ.get("DMP_L0_AT", "12"))
L1_ONLY = int(os.environ.get("DMP_L1_ONLY", "0"))  # debug: time bare L1 loop

_TANH = mybir.ActivationFunctionType.Tanh


def _round_fp32r(x: np.ndarray) -> np.ndarray:
    """Round fp32 -> fp32r (11 explicit mantissa bits), nearest-even."""
    b = np.ascontiguousarray(x, dtype=np.float32).view(np.uint32)
    lsb = (b >> np.uint32(12)) & np.uint32(1)
    r = b + (np.uint32(0x7FF) + lsb)
    r &= np.uint32(0xFFFFF000)
    return r.view(np.float32)


def _bf16(x: np.ndarray) -> np.ndarray:
    return np.ascontiguousarray(np.asarray(x, np.float32)).astype(ml_dtypes.bfloat16)


def _dmp_coefficients():
    """Closed-form coefficients of the sampled-position differences.

    Returns (d_alpha, d_beta, d_gamma, dQ) with dQ shaped (NOUT, N_BASIS):
      out[r, j] = d_alpha[j]*y0 + d_beta[j]*dy0 + d_gamma[j]*goal
                  + (goal - y0) * sum_n w[r, n] * dQ[j, n]
    """
    A_X, A_Z, TAU, DT = 1.0, 25.0, 1.0, 0.01
    B_Z = A_Z / 4.0
    NSTEP, L_SUB = 100, 10

    c = np.exp(-A_X * np.linspace(0.0, 1.0, N_BASIS))
    h = N_BASIS ** 1.5 / c / A_X
    xs = (1.0 - A_X * DT / TAU) ** np.arange(1, NSTEP + 1)
    psi = np.exp(-h[None, :] * (xs[:, None] - c[None, :]) ** 2)
    p = psi * xs[:, None] / psi.sum(axis=1, keepdims=True)      # (100, 5)

    nb = 3 + NSTEP
    cy = np.zeros(nb)
    cz = np.zeros(nb)
    cy[0] = 1.0
    cz[1] = TAU
    ys = [cy.copy()]
    for k in range(NSTEP):
        dz = np.zeros(nb)
        dz[2] = A_Z * B_Z
        dz -= A_Z * B_Z * cy
        dz -= A_Z * cz
        dz[3 + k] += 1.0
        dz /= TAU
        dy = cz / TAU
        cy = cy + dy * DT
        cz = cz + dz * DT
        ys.append(cy.copy())
    ys = np.array(ys)                         # (101, 103)
    samp = ys[::L_SUB]                        # (11, 103)
    dcoef = samp[1:] - samp[:-1]              # (10, 103)
    dQ = dcoef[:, 3:] @ p                     # (10, 5)
    return dcoef[:, 0], dcoef[:, 1], dcoef[:, 2], dQ


_NC_CACHE = {}


def _build_program(tw: int, repeat: int, fori_reps: int = 1, stagger: int = STAGGER):
    nt = B_SH // tw
    # one h0 generation per batch tile, written two tiles ahead; the
    # staggered-reset stage-adjacency invariant (stage I waits on I-2)
    # makes the cross-iteration handoff race-free only at distance 2
    assert nt == 4, f"nt={nt} must be 4"
    nc = bacc.Bacc()

    xb = nc.dram_tensor("xb", [D_IN, B_SH], BF16, kind="ExternalInput")
    ydy = nc.dram_tensor("ydy", [18, B_SH], BF16, kind="ExternalInput")
    w0t = nc.dram_tensor("w0t", [D_IN, H], BF16, kind="ExternalInput")
    b0d = nc.dram_tensor("b0d", [128, HC], F32, kind="ExternalInput")
    w1t = nc.dram_tensor("w1t", [H, H], BF16, kind="ExternalInput")
    b1d = nc.dram_tensor("b1d", [128, HC], F32, kind="ExternalInput")
    weff = nc.dram_tensor("weff", [H, M_ALL], BF16, kind="ExternalInput")
    beff = nc.dram_tensor("beff", [M_ALL, 1], F32, kind="ExternalInput")
    linc = nc.dram_tensor("linc", [117, M_S], BF16, kind="ExternalInput")
    diffc = nc.dram_tensor("diffc", [117, M_S], BF16, kind="ExternalInput")
    outT = nc.dram_tensor("outT", [M_S, B_SH], F32, kind="ExternalOutput")

    with TileContext(nc) as tc:
        with (
            tc.tile_pool(name="wres", bufs=1) as wres,
            tc.tile_pool(name="h1p", bufs=3) as h1p,
            tc.tile_pool(name="outp", bufs=4) as outp,
            tc.tile_pool(name="ps_l0", bufs=2, space="PSUM") as ps_l0,
            tc.tile_pool(name="ps_h1", bufs=4, space="PSUM") as ps_h1,
            tc.tile_pool(name="ps_m", bufs=2, space="PSUM") as ps_m,
        ):
            # ---- resident weights / constants / input ----
            w0_sb = wres.tile([128, H], BF16, tag="w0")
            nc.sync.dma_start(out=w0_sb, in_=w0t[:, :])
            b0_sb = wres.tile([128, HC], F32, tag="b0")
            nc.sync.dma_start(out=b0_sb, in_=b0d[:, :])
            x_all = wres.tile([128, B_SH], BF16, tag="xall")
            nc.sync.dma_start(out=x_all, in_=xb[:, :])
            b1_sb = wres.tile([128, HC], F32, tag="b1")
            nc.sync.dma_start(out=b1_sb, in_=b1d[:, :])
            # j-major 128x128 blocks: the layer-1 j-loop consumes w1[:, :, j*128]
            # column blocks in order, so tile 0's j-loop can start as soon as
            # the first blocks land instead of waiting for the whole 8 MB
            w1_sb = wres.tile([128, HC, H], BF16, tag="w1")
            for j in range(HC):
                for i in range(HC):
                    nc.sync.dma_start(
                        out=w1_sb[:, i, j * 128:(j + 1) * 128],
                        in_=w1t[i * 128:(i + 1) * 128, j * 128:(j + 1) * 128],
                    )
            weff_sb = wres.tile([128, HC, M_ALL], BF16, tag="weff")
            for i in range(HC):
                nc.sync.dma_start(out=weff_sb[:, i, :], in_=weff[i * 128:(i + 1) * 128, :])
            beff_sb = wres.tile([M_ALL, 1], F32, tag="beff")
            nc.sync.dma_start(out=beff_sb, in_=beff[:, :])
            linc_sb = wres.tile([117, M_S], BF16, tag="linc")
            nc.sync.dma_start(out=linc_sb, in_=linc[:, :])
            diffc_sb = wres.tile([117, M_S], BF16, tag="diffc")
            nc.sync.dma_start(out=diffc_sb, in_=diffc[:, :])

            # persistent h0 double generation + per-tile m2 combine tiles
            h0g = [
                [
                    wres.tile(
                        [128, tw], BF16, tag=f"h0_{g}_{c}", name=f"h0_{g}_{c}"
                    )
                    for c in range(HC)
                ]
                for g in range(nt)
            ]
            m2t = [
                wres.tile([117, tw], BF16, tag=f"m2_{t}", name=f"m2_{t}")
                for t in range(nt)
            ]
            for t in range(nt):
                win = slice(t * tw, (t + 1) * tw)
                nc.sync.dma_start(out=m2t[t][99:117, :], in_=ydy[:, win])

            def l0_mm(t_target, c):
                """Layer-0 matmul for chunk c of batch tile t_target."""
                win = slice(t_target * tw, (t_target + 1) * tw)
                ps = ps_l0.tile([128, tw], F32, tag="l0")
                nc.tensor.matmul(
                    ps, w0_sb[:, c * 128:(c + 1) * 128], x_all[:, win],
                    start=True, stop=True,
                )
                return ps

            def l0_act(ps, dst, c):
                nc.scalar.activation(
                    out=dst, in_=ps, func=_TANH, bias=b0_sb[:, c:c + 1],
                )

            def l0_chunk(dst, t_target, c):
                """h0 chunk c for batch tile t_target -> persistent tile dst."""
                l0_act(l0_mm(t_target, c), dst, c)

            # preamble: generations 0/1 = tiles 0/1's h0
            for c in range(HC):
                l0_chunk(h0g[0][c], 0, c)
            for c in range(HC):
                l0_chunk(h0g[1][c], 1, c)
            if L1_ONLY:
                for g in (2, 3):
                    for c in range(HC):
                        l0_chunk(h0g[g][c], g, c)

            def tail(t):
                win = slice(t * tw, (t + 1) * tw)
                m2 = m2t[t]
                lin_ps = ps_l0.tile([M_S, tw], F32, tag="l0")
                nc.tensor.matmul(lin_ps, linc_sb, m2[0:117, :], start=True, stop=True)
                diff_ps = ps_l0.tile([M_S, tw], F32, tag="l0")
                nc.tensor.matmul(diff_ps, diffc_sb, m2[0:117, :], start=True, stop=True)
                prod = outp.tile([M_S, tw], F32, tag="prod")
                nc.vector.tensor_mul(prod, diff_ps, m2[0:M_S, :])
                res = outp.tile([M_S, tw], F32, tag="res")
                nc.vector.tensor_add(res, prod, lin_ps)
                nc.sync.dma_start(out=outT[:, win], in_=res)

            # software-pipeline state; carried across passes in the
            # straight-line repeat path so a pass's trailing weff/bias/tail
            # overlap the next pass's first groups (flushed per-iteration in
            # the For_i path, which must be self-contained)
            psm_tiles = {}
            pend_weff = []   # (t, j, h1c) awaiting weff issue
            pend_bias = []   # tiles whose psm group closed, bias not issued
            pend_tail = []   # tiles whose bias issued, tail not issued

            def _one_pass(staged=False, flush=True, boundaries=None, g0=0):
                # boundaries: set of global tile indices that begin a new
                # stagger stage (the reset machinery needs exactly 4 stages
                # per For_i body); g0 = this pass's first global tile index
                # software pipeline: the weff matmul for group (t, j) is
                # issued WEFF_AT matmuls into group j+1 so the in-order PE
                # queue never waits on the ACT tanh that produces h1c.
                # Likewise the m2 bias-add (DVE) and the tail matmuls.

                def issue_weff():
                    t_, j_, h1c_ = pend_weff.pop(0)
                    if j_ == 0:
                        psm_tiles[t_] = ps_m.tile(
                            [M_ALL, tw], F32, tag="m", name=f"psm_{t_}"
                        )
                    nc.tensor.matmul(
                        psm_tiles[t_], weff_sb[:, j_, :], h1c_,
                        start=(j_ == 0), stop=(j_ == HC - 1),
                        skip_group_check=True,
                    )
                    if j_ == HC - 1:
                        pend_bias.append(t_)

                def issue_bias():
                    t_ = pend_bias.pop(0)
                    nc.vector.tensor_scalar_add(
                        out=m2t[t_][0:M_ALL, :],
                        in0=psm_tiles.pop(t_),
                        scalar1=beff_sb[:, 0:1],
                    )
                    pend_tail.append(t_)

                for t in range(nt):
                    if staged and (boundaries is None or (g0 + t) in boundaries):
                        if boundaries is not None or t > 0:
                            tc.stage_boundary()
                    gen = h0g[t]
                    ngen = h0g[(t + 2) % nt]
                    nxt = (t + 2) % nt
                    for j in range(HC):
                        ps1 = ps_h1.tile([128, tw], F32, tag="h1")
                        for i in range(HC):
                            nc.tensor.matmul(
                                ps1, w1_sb[:, i, j * 128:(j + 1) * 128], gen[i],
                                start=(i == 0), stop=(i == HC - 1),
                            )
                            if L1_ONLY:
                                continue
                            if i == WEFF_AT and pend_weff:
                                issue_weff()
                            if i == BIAS_AT and pend_bias:
                                issue_bias()
                            if i == TAIL_AT and pend_tail and not (t == 0 and j == 0):
                                tail(pend_tail.pop(0))
                            if i == L0_AT and not L1_ONLY:
                                # layer-0 for tile t+2 (wraps into the next
                                # pass for t >= 2; x is identical every pass);
                                # MM and tanh both issued here: the early l0
                                # tanh warms the ACT queue during the group
                                # tail and frees the shared ps_l0 bank before
                                # the tail matmuls allocate from it (delaying
                                # it measured 300-305us vs 294-298us)
                                l0_chunk(ngen[j], nxt, j)
                        h1c = h1p.tile([128, tw], BF16, tag="h1c")
                        nc.scalar.activation(
                            out=h1c, in_=ps1, func=_TANH, bias=b1_sb[:, j:j + 1],
                        )
                        if L1_ONLY:
                            continue
                        pend_weff.append((t, j, h1c))
                # end-of-pass flush (last tile's final weff, bias, tail);
                # skipped mid-stream in the repeat path so the next pass's
                # first groups absorb the latency
                if flush:
                    while pend_weff:
                        issue_weff()
                    while pend_bias:
                        issue_bias()
                    while pend_tail:
                        tail(pend_tail.pop(0))

            if fori_reps > 1:
                ntiles = repeat * nt
                assert ntiles % 4 == 0
                q = ntiles // 4
                bset = {q, 2 * q, 3 * q}
                with tc.For_i(
                    0, fori_reps, 1,
                    hint_engines=(mybir.EngineType.PE,),
                    staggered_reset=bool(stagger),
                ):
                    for _rep in range(repeat):
                        _one_pass(
                            staged=bool(stagger),
                            flush=(_rep == repeat - 1),
                            boundaries=bset,
                            g0=_rep * nt,
                        )
            else:
                for _rep in range(repeat):
                    _one_pass(flush=(_rep == repeat - 1))

    nc.compile()
    return nc


def _get_program(tw: int = TW, repeat: int = REPEAT, fori_reps: int = FORI_REPS):
    key = (tw, repeat, fori_reps, STAGGER)
    if key not in _NC_CACHE:
        _NC_CACHE[key] = _build_program(tw, repeat, fori_reps, STAGGER)
    return _NC_CACHE[key]


def _prepare_host_inputs(input, W0, b0, W1, b1, Wl, bl):
    """Build the per-core input maps (host-side prep, float64 coefficients)."""
    input, W0, b0, W1, b1, Wl, bl = (
        np.asarray(a) for a in (input, W0, b0, W1, b1, Wl, bl)
    )
    d_alpha, d_beta, d_gamma, dQ = _dmp_coefficients()

    Wl100 = Wl.astype(np.float64) * 100.0          # (54, H)
    bl100 = bl.astype(np.float64) * 100.0          # (54,)

    # effective final layer: rows 0..89 = S rows (d*10+j), 90..98 = goal rows
    weff = np.zeros((H, M_ALL), dtype=np.float64)
    beff = np.zeros((M_ALL,), dtype=np.float64)
    for d in range(DIM):
        for j in range(NOUT):
            m = d * NOUT + j
            wrow = np.zeros(H, dtype=np.float64)
            brow = 0.0
            for n in range(N_BASIS):
                wrow += dQ[j, n] * Wl100[DIM + N_BASIS * d + n]
                brow += dQ[j, n] * bl100[DIM + N_BASIS * d + n]
            weff[:, m] = wrow
            beff[m] = brow
        weff[:, M_S + d] = Wl100[d]
        beff[M_S + d] = bl100[d]

    # broadcast matmul constants [117, 90]: rhs is the combined mlp2 tile
    # (rows 0..89 = S [zero coeff], 90..98 = goal, 99..107 = y0, 108..116 = dy0)
    linc = np.zeros((117, M_S), dtype=np.float64)
    diffc = np.zeros((117, M_S), dtype=np.float64)
    for d in range(DIM):
        for j in range(NOUT):
            m = d * NOUT + j
            linc[90 + d, m] = d_gamma[j]
            linc[99 + d, m] = d_alpha[j]
            linc[108 + d, m] = d_beta[j]
            diffc[90 + d, m] = 1.0
            diffc[99 + d, m] = -1.0

    shared = {
        "w0t": _bf16(W0.T),
        "b0d": np.ascontiguousarray(np.asarray(b0, np.float32).reshape(HC, 128).T),
        "w1t": _bf16(W1.T),
        "b1d": np.ascontiguousarray(np.asarray(b1, np.float32).reshape(HC, 128).T),
        "weff": _bf16(weff),
        "beff": np.ascontiguousarray(beff.astype(np.float32).reshape(M_ALL, 1)),
        "linc": _bf16(linc),
        "diffc": _bf16(diffc),
    }

    x32 = np.asarray(input, np.float32)
    in_maps = []
    for c in range(N_CORES):
        m = dict(shared)
        xc = x32[c * B_SH:(c + 1) * B_SH, :]
        m["xb"] = _bf16(xc.T)
        m["ydy"] = _bf16(np.concatenate([xc[:, 7:16], xc[:, 22:31]], axis=1).T)
        in_maps.append(m)
    return in_maps


def kernel(input, W0, b0, W1, b1, Wl, bl):
    nc = _get_program()
    in_maps = _prepare_host_inputs(input, W0, b0, W1, b1, Wl, bl)
    results = run_bass_kernel_spmd(nc, in_maps, core_ids=list(range(N_CORES)))
    outs = []
    for c in range(N_CORES):
        o = results.results[c]["outT"]                     # (90, 2048)
        outs.append(o.reshape(DIM, NOUT, B_SH).transpose(2, 0, 1))
    return np.ascontiguousarray(np.concatenate(outs, axis=0), dtype=np.float32)

